# revision 1
# baseline (speedup 1.0000x reference)
"""NNCLR forward loss kernel for 8x TRN2 NeuronCores.

Strategy (hint-aligned): shard feature_queue rows across the 8 cores.
Each core computes sims = proj @ queue_shard.T for both projections
(1024 total rows) with fp32r matmuls, finds per-row shard max+argmax
(exact fp32 compare semantics, first-occurrence ties), AllGathers the
(max, argmax) pairs, selects the global winner per row, gathers the
winning queue rows by indirect DMA (owner core) + AllReduce(add), then
every core computes the 4 BxB logit matrices, log-softmax diagonals
and the final [4B] loss (replicated; host takes core 0's copy).
"""

import numpy as np

import concourse.bass as bass
import concourse.mybir as mybir
from concourse.bass import IndirectOffsetOnAxis
from concourse.tile import TileContext

import bass_rust as _br
import concourse.tile as _tile_mod


def _patched_drain_and_barrier(self, tick_clock, wait_clock):
    """Walrus here only allows 2 sem waits per instruction; split the
    Tile tail drain's wait list across extra drain instructions."""
    drain_inst = self.nc.sync.drain()
    wait_clock.add_sem_waits(
        drain_inst.ins, _br.ScopedClock({None: tick_clock.global_clock})
    )
    si = drain_inst.ins.sync_info
    if si is not None and si.on_wait and len(si.on_wait) > 1:
        waits = list(si.on_wait)
        drain_inst.ins.sync_info = _br.SyncInfo(on_wait=waits[:1], on_update=list(si.on_update))
        for i in range(1, len(waits)):
            extra = self.nc.sync.drain()
            extra.ins.sync_info = _br.SyncInfo(on_wait=waits[i : i + 1], on_update=[])
    self.nc.all_engine_barrier()
    assert self.sems is not None
    popped = self.nc._tile_sem_poison_stack.pop()
    assert popped is self._sem_poison
    self.nc.clear_and_free_semaphores(list(self.sems.allocated().values()))
    self.nc.all_engine_barrier()


_tile_mod.TileContext._drain_and_barrier = _patched_drain_and_barrier


def _split_multi_waits(nc):
    """This walrus build allows only one sync-wait per instruction; hoist
    extra waits onto NOPs inserted just before, on the same engine."""
    n_split = 0
    for f in nc.m.functions:
        for bb in f.blocks:
            il = bb.instructions
            i = 0
            while i < len(il):
                inst = il[i]
                si = inst.sync_info
                if si is not None and si.on_wait and len(si.on_wait) > 1:
                    waits = list(si.on_wait)
                    nops = []
                    for w in waits[:-1]:
                        nop = mybir.InstNoOp(
                            name=f"waitsplit-{nc.next_id()}",
                            engine=inst.engine,
                            ins=[],
                            outs=[],
                            sync_info=_br.SyncInfo(on_wait=[w], on_update=[]),
                        )
                        nc.register_instruction(nop, overwrite=True)
                        nops.append(nop)
                    inst.sync_info = _br.SyncInfo(
                        on_wait=[waits[-1]], on_update=list(si.on_update)
                    )
                    il[i:i] = nops
                    i += len(nops)
                    n_split += 1
                i += 1
    return n_split

F32 = mybir.dt.float32
F32R = mybir.dt.float32r
U16 = mybir.dt.uint16
U32 = mybir.dt.uint32

B = 512  # rows per projection
D = 256  # feature dim
B2 = 2 * B  # 1024 combined rows (p1 then p2)
NCORES = 8
Q_FULL = 98304
QS = Q_FULL // NCORES  # 12288 queue rows per core
CHUNK = 512
AF = mybir.ActivationFunctionType
ALL_CORES = [list(range(NCORES))]


def build_nc_A(qs=QS, use_f32r=False):
    """Launch A: per-core sims + exact shard max/argmax. Inputs p1T/p2T/qT."""
    nch = qs // CHUNK
    nt = B2 // 128
    nc = bass.Bass(num_devices=NCORES, debug=False)
    p1T = nc.declare_dram_parameter("p1T", [D, B], F32, isOutput=False)
    p2T = nc.declare_dram_parameter("p2T", [D, B], F32, isOutput=False)
    qT = nc.declare_dram_parameter("qT", [D, qs], F32, isOutput=False)
    mj_out = nc.declare_dram_parameter("mj", [128, 16], F32, isOutput=True)

    def mmcast(ap):
        return ap.bitcast(F32R) if use_f32r else ap

    with TileContext(nc) as tc:
        with (
            tc.tile_pool(name="persist", bufs=1) as pp,
            tc.tile_pool(name="qchunk", bufs=3) as qpool,
            tc.tile_pool(name="sims", bufs=2) as simpool,
            tc.tile_pool(name="small", bufs=4) as sp,
            tc.tile_pool(name="psumA", bufs=6, space="PSUM") as psA,
        ):
            pT_all = pp.tile([128, 2, B2], F32)
            nc.sync.dma_start(pT_all[:, :, 0:B], p1T.ap().rearrange("(k p) b -> p k b", p=128))
            nc.sync.dma_start(pT_all[:, :, B:B2], p2T.ap().rearrange("(k p) b -> p k b", p=128))

            m_all = pp.tile([128, nt], F32)
            jf_all = pp.tile([128, nt], F32)
            qT3 = qT.ap().rearrange("(k p) q -> p k q", p=128)
            for pr in range(nt // 2):
                sims_t = [
                    simpool.tile([128, qs], F32, tag="sims", name=f"sims_{pr}_{ti}")
                    for ti in range(2)
                ]
                for c in range(nch):
                    qt = qpool.tile([128, 2, CHUNK], F32)
                    nc.sync.dma_start(qt[:], qT3[:, :, c * CHUNK : (c + 1) * CHUNK])
                    for ti in range(2):
                        t = pr * 2 + ti
                        ps = psA.tile([128, CHUNK], F32)
                        nc.tensor.matmul(
                            ps[:],
                            mmcast(pT_all[:, 0, t * 128 : (t + 1) * 128]),
                            mmcast(qt[:, 0, :]),
                            start=True, stop=False,
                        )
                        nc.tensor.matmul(
                            ps[:],
                            mmcast(pT_all[:, 1, t * 128 : (t + 1) * 128]),
                            mmcast(qt[:, 1, :]),
                            start=False, stop=True,
                        )
                        nc.scalar.copy(sims_t[ti][:, c * CHUNK : (c + 1) * CHUNK], ps[:])
                for ti in range(2):
                    t = pr * 2 + ti
                    top8 = sp.tile([128, 8], F32)
                    nc.vector.max(top8[:], sims_t[ti][:])
                    idx8 = sp.tile([128, 8], U32)
                    nc.vector.max_index(idx8[:], top8[:], sims_t[ti][:])
                    nc.vector.tensor_copy(m_all[:, t : t + 1], top8[:, 0:1])
                    nc.vector.tensor_copy(jf_all[:, t : t + 1], idx8[:, 0:1])

            pack = pp.tile([128, 16], F32)
            nc.vector.tensor_copy(pack[:, 0:8], m_all[:])
            nc.vector.tensor_copy(pack[:, 8:16], jf_all[:])
            nc.sync.dma_start(mj_out.ap(), pack[:])

    _split_multi_waits(nc)
    return nc


def build_nc_C(use_f32r=False):
    """Launch C: logits + log-softmax loss from host-gathered nn rows."""
    nt = B2 // 128
    nc = bass.Bass(num_devices=NCORES, debug=False)
    p1 = nc.declare_dram_parameter("p1", [B, D], F32, isOutput=False)
    p2 = nc.declare_dram_parameter("p2", [B, D], F32, isOutput=False)
    nn_in = nc.declare_dram_parameter("nn", [128, nt, D], F32, isOutput=False)
    temp = nc.declare_dram_parameter("temp", [1, 1], F32, isOutput=False)
    loss_out = nc.declare_dram_parameter("loss", [16, 128], F32, isOutput=True)

    def mmcast(ap):
        return ap.bitcast(F32R) if use_f32r else ap

    with TileContext(nc) as tc:
        with (
            tc.tile_pool(name="persist", bufs=1) as pp,
            tc.tile_pool(name="small", bufs=2) as sp,
        ):
            p_nat = pp.tile([128, nt, D], F32)
            nc.sync.dma_start(p_nat[:, 0 : nt // 2, :], p1.ap().rearrange("(t p) d -> p t d", p=128))
            nc.sync.dma_start(p_nat[:, nt // 2 : nt, :], p2.ap().rearrange("(t p) d -> p t d", p=128))
            nn_full = pp.tile([128, nt, D], F32)
            nc.sync.dma_start(nn_full[:], nn_in.ap())

            t128 = pp.tile([128, 1], F32)
            nc.sync.dma_start(t128[:], temp.ap().to_broadcast((128, 1)))
            itb = pp.tile([128, 1], F32)
            nc.vector.reciprocal(itb[:], t128[:])

            sq = sp.tile([128, nt, D], F32, bufs=1)
            nc.vector.tensor_mul(sq[:], p_nat[:], p_nat[:])
            n2 = pp.tile([128, nt], F32)
            nc.vector.reduce_sum(n2[:], sq[:], axis=mybir.AxisListType.X)
            nrm = pp.tile([128, nt], F32)
            nc.scalar.sqrt(nrm[:], n2[:])
            nc.vector.tensor_scalar_max(nrm[:], nrm[:], 1e-12)
            inv = pp.tile([128, nt], F32)
            nc.vector.reciprocal(inv[:], nrm[:])
            invs = pp.tile([128, nt], F32)
            nc.vector.tensor_mul(invs[:], inv[:], itb[:, 0:1].to_broadcast((128, nt)))
            p_norm = pp.tile([128, nt, D], F32)
            nc.vector.tensor_mul(p_norm[:], p_nat[:], inv[:, :, None].to_broadcast((128, nt, D)))
            p_scal = pp.tile([128, nt, D], F32)
            nc.vector.tensor_mul(p_scal[:], p_nat[:], invs[:, :, None].to_broadcast((128, nt, D)))

            nn_adj = pp.tile([128, nt, D], F32)
            nc.vector.tensor_sub(nn_adj[:], nn_full[:], p_norm[:])
            nc.vector.tensor_add(nn_adj[:], p_norm[:], nn_adj[:])

            ident_dram = nc.inline_tensor(np.eye(128, dtype=np.float32), name="ident128")
            ident = pp.tile([128, 128], F32)
            nc.sync.dma_start(ident[:], ident_dram.ap())

            nnT = pp.tile([128, 2, B2], F32)
            psT = pp.tile([128, 2, B2], F32)
            with tc.tile_pool(name="psumT", bufs=4, space="PSUM") as psT_pool:
                for t in range(nt):
                    for kblk in range(2):
                        ptile = psT_pool.tile([128, 128], F32, tag="tp")
                        nc.tensor.transpose(ptile[:], nn_adj[:, t, kblk * 128 : (kblk + 1) * 128], ident[:])
                        nc.vector.tensor_copy(nnT[:, kblk, t * 128 : (t + 1) * 128], ptile[:])
                        ptile2 = psT_pool.tile([128, 128], F32, tag="tp")
                        nc.tensor.transpose(ptile2[:], p_scal[:, t, kblk * 128 : (kblk + 1) * 128], ident[:])
                        nc.scalar.copy(psT[:, kblk, t * 128 : (t + 1) * 128], ptile2[:])

            h = nt // 2
            dmul = sp.tile([128, nt, D], F32, tag="dmul", bufs=1)
            nc.vector.tensor_mul(dmul[:, 0:h, :], nn_adj[:, 0:h, :], p_scal[:, h:nt, :])
            nc.vector.tensor_mul(dmul[:, h:nt, :], nn_adj[:, h:nt, :], p_scal[:, 0:h, :])
            dg = pp.tile([128, nt], F32)
            nc.vector.reduce_sum(dg[:], dmul[:], axis=mybir.AxisListType.X)

            sl = pp.tile([128, 16], F32)
            nc.vector.tensor_copy(sl[:, 0:4], dg[:, 0:4])
            nc.vector.tensor_copy(sl[:, 4:8], dg[:, 0:4])
            nc.vector.tensor_copy(sl[:, 8:12], dg[:, 4:8])
            nc.vector.tensor_copy(sl[:, 12:16], dg[:, 4:8])

            Mall = pp.tile([128, 16], F32)
            negM = pp.tile([128, 16], F32)
            Sall = pp.tile([128, 16], F32)
            nn1T = nnT[:, :, 0:B]
            nn2T = nnT[:, :, B:B2]
            p1sT = psT[:, :, 0:B]
            p2sT = psT[:, :, B:B2]
            with tc.tile_pool(name="psumC", bufs=8, space="PSUM") as psC_pool:
                for rt in range(16):
                    mat = rt // 4
                    i = rt % 4
                    if mat == 0:
                        lhs, rhs = nn1T, p2sT
                    elif mat == 1:
                        lhs, rhs = p2sT, nn1T
                    elif mat == 2:
                        lhs, rhs = nn2T, p1sT
                    else:
                        lhs, rhs = p1sT, nn2T
                    psc = psC_pool.tile([128, B], F32)
                    for kblk in range(2):
                        nc.tensor.matmul(
                            psc[:],
                            mmcast(lhs[:, kblk, i * 128 : (i + 1) * 128]),
                            mmcast(rhs[:, kblk, :]),
                            start=(kblk == 0), stop=(kblk == 1),
                        )
                    nc.vector.reduce_max(Mall[:, rt : rt + 1], psc[:], axis=mybir.AxisListType.X)
                    nc.vector.tensor_scalar_mul(negM[:, rt : rt + 1], Mall[:, rt : rt + 1], -1.0)
                    escr = sp.tile([128, B], F32, tag="escr", bufs=2)
                    nc.scalar.activation(
                        escr[:], psc[:], AF.Exp,
                        bias=negM[:, rt : rt + 1], scale=1.0,
                        accum_out=Sall[:, rt : rt + 1],
                    )

            lnS = pp.tile([128, 16], F32)
            nc.scalar.activation(lnS[:], Sall[:], AF.Ln)
            lossT = pp.tile([128, 16], F32)
            nc.vector.tensor_add(lossT[:], lnS[:], Mall[:])
            nc.vector.tensor_sub(lossT[:], lossT[:], sl[:])
            nc.sync.dma_start(loss_out.ap().rearrange("rt p -> p rt"), lossT[:])

    _split_multi_waits(nc)
    return nc


_CACHE = {}


def _get_nc(which, use_f32r=False):
    key = (which, use_f32r)
    if key not in _CACHE:
        _CACHE[key] = build_nc_A(use_f32r=use_f32r) if which == "A" else build_nc_C(use_f32r=use_f32r)
    return _CACHE[key]


def kernel(projections_1, projections_2, feature_queue, temperature):
    from concourse.bass_utils import run_bass_kernel_spmd

    p1 = np.ascontiguousarray(projections_1, dtype=np.float32)
    p2 = np.ascontiguousarray(projections_2, dtype=np.float32)
    fq = np.ascontiguousarray(feature_queue, dtype=np.float32)
    t = np.array(temperature, dtype=np.float32).reshape(1, 1)
    p1T = np.ascontiguousarray(p1.T)
    p2T = np.ascontiguousarray(p2.T)

    # ---- launch A: sharded sims + per-core exact top-1 ----
    ncA = _get_nc("A")
    in_maps = []
    for c in range(NCORES):
        shard = fq[c * QS : (c + 1) * QS]
        in_maps.append({"p1T": p1T, "p2T": p2T, "qT": np.ascontiguousarray(shard.T)})
    resA = run_bass_kernel_spmd(ncA, in_maps, core_ids=list(range(NCORES)))
    mj = np.stack([np.asarray(resA.results[c]["mj"]) for c in range(NCORES)])  # [8, 128, 16]
    # row r = t*128 + p lives at mj[c, p, t] / mj[c, p, 8+t]
    m_g = mj[:, :, 0:8].transpose(0, 2, 1).reshape(NCORES, B2)  # [core, row]
    j_g = mj[:, :, 8:16].transpose(0, 2, 1).reshape(NCORES, B2)
    wc = np.argmax(m_g, axis=0)  # first-occurrence ties -> lowest core, matching global argmax
    jglob = wc * QS + j_g[wc, np.arange(B2)].astype(np.int64)
    nn = fq[jglob]  # [1024, 256]

    # ---- launch C: logits + loss on one core ----
    ncC = _get_nc("C")
    nn_dev = np.ascontiguousarray(nn.reshape(8, 128, D).transpose(1, 0, 2))
    resC = run_bass_kernel_spmd(
        ncC, [{"p1": p1, "p2": p2, "nn": nn_dev, "temp": t}], core_ids=[0]
    )
    loss = np.asarray(resC.results[0]["loss"], dtype=np.float32).reshape(-1)
    return loss

